# revision 2
# baseline (speedup 1.0000x reference)
"""Multi-head attention (B=4, S=2048, d_model=1024, h=16) on 8 TRN2 NeuronCores.

Sharding: data-parallel over batch (4) x tensor-parallel over head-groups (2 x 8
heads, column-split Wq/Wk/Wv, row-split Wo). Each core computes a full (2048,
1024) partial of the output projection for its (batch, head-group); the host
sums the two group partials per batch and adds bo.

Device kernel (identical SPMD program on all 8 cores):
  qT/kT = W @ X.T computed directly in head-major layout (TF32 matmuls, PE at
  full rate), scoresT = k @ qT per head with 64x128 row-tiled matmul pairs (two
  heads run concurrently on the two halves of the PE array), one 2048-wide exp
  per 4-bank PSUM block on the scalar engine (the per-instruction overhead
  makes narrow activations 40% slower), AV as [v|1].T @ exps so the softmax
  denominators fall out of the matmul for free, normalization via vector-engine
  reciprocal + gpsimd partition-broadcast, then the output projection from the
  already-transposed attention output.
"""
import numpy as np

import concourse.bacc as bacc
import concourse.mybir as mybir
from concourse.tile import TileContext
from concourse.bass_utils import run_bass_kernel_spmd

P = 128
S = 2048          # sequence length
DM = 1024         # d_model
DG = 512          # dims per head-group (8 heads x 64)
NPAIR = 4         # head pairs per group
NQB = 4           # q blocks of 512
NKT = 16          # key tiles of 128
KT = DM // P      # contraction tiles for projections

F32 = mybir.dt.float32
F32R = mybir.dt.float32r
BF16 = mybir.dt.bfloat16
AF = mybir.ActivationFunctionType


def _build(has_bias):
    nc = bacc.Bacc(None, target_bir_lowering=False)
    xqT = nc.dram_tensor("xqT", [DM, S], F32R, kind="ExternalInput")
    xkT = nc.dram_tensor("xkT", [DM, S], F32R, kind="ExternalInput")
    xvT = nc.dram_tensor("xvT", [DM, S], F32R, kind="ExternalInput")
    wqT = nc.dram_tensor("wqT", [DM, DG], F32R, kind="ExternalInput")
    wkT = nc.dram_tensor("wkT", [DM, DG], F32R, kind="ExternalInput")
    wvT = nc.dram_tensor("wvT", [DM, DG], F32R, kind="ExternalInput")
    woT = nc.dram_tensor("woT", [DG, DM], F32R, kind="ExternalInput")
    if has_bias:
        bq = nc.dram_tensor("bq", [1, DG], F32R, kind="ExternalInput")
        bk = nc.dram_tensor("bk", [1, DG], F32R, kind="ExternalInput")
        bv = nc.dram_tensor("bv", [1, DG], F32R, kind="ExternalInput")
    out = nc.dram_tensor("out", [S, DM], F32, kind="ExternalOutput")

    xT = {"q": xqT, "k": xkT, "v": xvT}
    wT = {"q": wqT, "k": wkT, "v": wvT}

    with TileContext(nc) as tc:
        with tc.tile_pool(name="pres", bufs=1) as pres, \
             tc.tile_pool(name="px", bufs=6) as px, \
             tc.tile_pool(name="pex", bufs=2) as pex, \
             tc.tile_pool(name="psmall", bufs=2) as psmall, \
             tc.tile_pool(name="pout", bufs=2) as pout, \
             tc.tile_pool(name="ps_proj", bufs=2, space="PSUM") as ps_proj, \
             tc.tile_pool(name="ps_sc", bufs=1, space="PSUM") as ps_sc, \
             tc.tile_pool(name="ps_av", bufs=2, space="PSUM") as ps_av:

            # resident tensors
            wq_sb = pres.tile([P, KT, DG], F32R)
            wk_sb = pres.tile([P, KT, DG], F32R)
            wv_sb = pres.tile([P, KT, DG], F32R)
            wo_sb = pres.tile([P, NPAIR, DM], F32R)
            qT_sb = pres.tile([P, NPAIR, S], BF16)
            kT_sb = pres.tile([P, NPAIR, S], BF16)
            v_sb = pres.tile([P, NKT, 8, 65], BF16)
            attn_sb = pres.tile([P, NPAIR, S], F32R)
            w_sb = {"q": wq_sb, "k": wk_sb, "v": wv_sb}

            for t, d in ((wq_sb, wqT), (wk_sb, wkT), (wv_sb, wvT)):
                nc.sync.dma_start(t[:], d.rearrange("(kt p) n -> p kt n", p=P))
            nc.sync.dma_start(wo_sb[:], woT.rearrange("(kp p) o -> p kp o", p=P))
            nc.vector.memset(v_sb[:, :, :, 64:65], 1.0)

            if has_bias:
                x9 = pres.tile([P, DG], F32R)      # ones row, rest zero
                xv9 = pres.tile([P, P], F32R)
                w9 = {
                    "q": pres.tile([P, DG], F32R, name="w9q"),
                    "k": pres.tile([P, DG], F32R, name="w9k"),
                    "v": pres.tile([P, DG], F32R, name="w9v"),
                }
                for t in (x9, xv9, w9["q"], w9["k"], w9["v"]):
                    nc.vector.memset(t[:], 0.0)
                nc.vector.memset(x9[0:1, :], 1.0)
                nc.vector.memset(xv9[0:1, :], 1.0)
                for key, d in (("q", bq), ("k", bk), ("v", bv)):
                    nc.sync.dma_start(w9[key][0:1, :], d[:])

            emitted = set()

            def qk_block(proj, nb):
                """Project q or k for seq block nb, all 4 pairs: fills
                {q,k}T_sb[:, :, nb*512:(nb+1)*512]."""
                dst = qT_sb if proj == "q" else kT_sb
                xs = []
                for j in range(KT // 2):
                    xt = px.tile([P, 2, DG], F32R, tag="x",
                                 name=f"x_{proj}{nb}_{j}")
                    nc.sync.dma_start(
                        xt[:],
                        xT[proj].rearrange("(kt p) s -> p kt s", p=P)
                        [:, 2 * j:2 * j + 2, nb * DG:(nb + 1) * DG],
                    )
                    xs.append(xt)
                for p in range(NPAIR):
                    ps = ps_proj.tile([P, DG], F32, tag="pp", name=f"ps_{proj}{nb}_{p}")
                    for kt in range(KT):
                        nc.tensor.matmul(
                            ps[:], w_sb[proj][:, kt, p * P:(p + 1) * P],
                            xs[kt // 2][:, kt % 2, :],
                            start=(kt == 0), stop=(kt == KT - 1 and not has_bias),
                        )
                    if has_bias:
                        nc.tensor.matmul(
                            ps[:], w9[proj][:, p * P:(p + 1) * P], x9[:],
                            start=False, stop=True,
                        )
                    nc.vector.tensor_copy(dst[:, p, nb * DG:(nb + 1) * DG], ps[:])

            def v_block(m):
                """Project v for key tile m (128 positions, all 8 heads):
                fills v_sb[:, m, :, 0:64]."""
                xt = px.tile([P, KT, P], F32R, tag="x", name=f"xv{m}")
                nc.sync.dma_start(
                    xt[:],
                    xvT.rearrange("(kt p) s -> p kt s", p=P)
                    [:, :, m * P:(m + 1) * P],
                )
                ps = ps_proj.tile([P, DG], F32, tag="pp", name=f"ps_v{m}")
                for kt in range(KT):
                    nc.tensor.matmul(
                        ps[:], xt[:, kt, :], wv_sb[:, kt, :],
                        start=(kt == 0), stop=(kt == KT - 1 and not has_bias),
                    )
                if has_bias:
                    nc.tensor.matmul(ps[:], xv9[:], w9["v"][:],
                                     start=False, stop=True)
                nc.vector.tensor_copy(
                    v_sb[:, m, :, 0:64],
                    ps[:].rearrange("p (h d) -> p h d", d=64),
                )

            def ensure(key):
                if key in emitted:
                    return
                emitted.add(key)
                kind, idx = key
                if kind == "v":
                    v_block(idx)
                else:
                    qk_block(kind, idx)

            def oproj_chunk(qb):
                """Output projection for seq rows qb*512 .. qb*512+512."""
                for mi in range(4):
                    m = 4 * qb + mi
                    for n in range(2):
                        ps = ps_proj.tile([P, DG], F32, tag="pp", name=f"ps_o{m}_{n}")
                        for kp in range(NPAIR):
                            nc.tensor.matmul(
                                ps[:], attn_sb[:, kp, m * P:(m + 1) * P],
                                wo_sb[:, kp, n * DG:(n + 1) * DG],
                                start=(kp == 0), stop=(kp == NPAIR - 1),
                            )
                        ot = pout.tile([P, DG], F32, tag="ot",
                                       name=f"ot{m}_{n}")
                        nc.vector.tensor_copy(ot[:], ps[:])
                        nc.sync.dma_start(
                            out[m * P:(m + 1) * P, n * DG:(n + 1) * DG], ot[:])

            # attention: pair-outer, q-block, then key-tile groups of 2
            for p in range(NPAIR):
                for qb in range(NQB):
                    ensure(("q", qb))
                    av = [
                        ps_av.tile([65, DG], F32, tag="av", name=f"av{p}_{qb}_{h}")
                        for h in range(2)
                    ]
                    for g in range(NKT // 2):
                        ensure(("k", g // 2))
                        ensure(("v", 2 * g))
                        ensure(("v", 2 * g + 1))
                        sc = ps_sc.tile([P, 2, 2, DG], F32, tag="sc",
                                        name=f"sc{p}_{qb}_{g}")
                        for h in range(2):
                            for i in range(2):
                                kt = 2 * g + i
                                nc.tensor.matmul(
                                    sc[:, h, i, :],
                                    kT_sb[64 * h:64 * h + 64, p,
                                          kt * P:(kt + 1) * P],
                                    qT_sb[64 * h:64 * h + 64, p,
                                          qb * DG:(qb + 1) * DG],
                                    start=True, stop=True,
                                    tile_position=(64 * h, 0),
                                )
                        ex = pex.tile([P, 2, 2, DG], BF16, tag="ex",
                                      name=f"ex{p}_{qb}_{g}")
                        nc.scalar.activation(ex[:], sc[:], AF.Exp, scale=0.125)
                        for h in range(2):
                            for i in range(2):
                                kt = 2 * g + i
                                nc.tensor.matmul(
                                    av[h][:],
                                    v_sb[:, kt, 2 * p + h, :],
                                    ex[:, h, i, :],
                                    start=(g == 0 and i == 0),
                                    stop=(g == NKT // 2 - 1 and i == 1),
                                )
                    for h in range(2):
                        rr = psmall.tile([1, DG], F32, tag="rr",
                                         name=f"rr{p}_{qb}_{h}")
                        nc.vector.reciprocal(rr[0:1, :], av[h][64:65, :])
                        rbc = psmall.tile([P, DG], F32, tag="rbc",
                                          name=f"rbc{p}_{qb}_{h}")
                        nc.gpsimd.partition_broadcast(rbc[:], rr[0:1, :])
                        nc.vector.tensor_tensor(
                            attn_sb[64 * h:64 * h + 64, p,
                                    qb * DG:(qb + 1) * DG],
                            av[h][0:64, :],
                            rbc[0:64, :],
                            mybir.AluOpType.mult,
                        )
                    if p == NPAIR - 1:
                        oproj_chunk(qb)
    nc.compile()
    return nc


_CACHE = {}


def _get_nc(has_bias):
    if has_bias not in _CACHE:
        _CACHE[has_bias] = _build(has_bias)
    return _CACHE[has_bias]


def _tr(a):
    return np.ascontiguousarray(np.asarray(a, dtype=np.float32).T)


def _run(Q, K, V, Wq, bq, Wk, bk, Wv, bv, Wo, bo, trace=False):
    Q, K, V = (np.asarray(t, np.float32) for t in (Q, K, V))
    Wq, Wk, Wv, Wo = (np.asarray(t, np.float32) for t in (Wq, Wk, Wv, Wo))
    bq, bk, bv, bo = (np.asarray(t, np.float32) for t in (bq, bk, bv, bo))
    B = Q.shape[0]
    has_bias = bool(np.any(bq) or np.any(bk) or np.any(bv))
    nc = _get_nc(has_bias)

    xts = [(_tr(Q[b]), _tr(K[b]), _tr(V[b])) for b in range(B)]
    wts = []
    for g in range(2):
        sl = slice(DG * g, DG * (g + 1))
        wts.append({
            "wqT": _tr(Wq[sl]), "wkT": _tr(Wk[sl]), "wvT": _tr(Wv[sl]),
            "woT": _tr(Wo[:, sl]),
            "bq": np.ascontiguousarray(bq[None, sl]),
            "bk": np.ascontiguousarray(bk[None, sl]),
            "bv": np.ascontiguousarray(bv[None, sl]),
        })
    in_maps = []
    for c in range(8):
        b, g = c // 2, c % 2
        m = {
            "xqT": xts[b][0], "xkT": xts[b][1], "xvT": xts[b][2],
            "wqT": wts[g]["wqT"], "wkT": wts[g]["wkT"],
            "wvT": wts[g]["wvT"], "woT": wts[g]["woT"],
        }
        if has_bias:
            m["bq"] = wts[g]["bq"]
            m["bk"] = wts[g]["bk"]
            m["bv"] = wts[g]["bv"]
        in_maps.append(m)

    res = run_bass_kernel_spmd(nc, in_maps, core_ids=list(range(8)),
                               trace=trace)
    outp = np.empty((B, S, DM), np.float32)
    for b in range(B):
        outp[b] = res.results[2 * b]["out"] + res.results[2 * b + 1]["out"]
    outp += bo[None, None, :]
    return outp, res


def kernel(Q, K, V, Wq, bq, Wk, bk, Wv, bv, Wo, bo):
    outp, _ = _run(Q, K, V, Wq, bq, Wk, bk, Wv, bv, Wo, bo, trace=False)
    return outp


# revision 3
# speedup vs baseline: 1.3351x; 1.3351x over previous
"""Multi-head attention (B=4, S=2048, d_model=1024, h=16) on 8 TRN2 NeuronCores.

Sharding: data-parallel over batch (4) x tensor-parallel over head-groups (2 x 8
heads, column-split Wq/Wk/Wv, row-split Wo). Each core computes a full (2048,
1024) partial of the output projection for its (batch, head-group); the host
sums the two group partials per batch and adds bo.

Device kernel (identical SPMD program on all 8 cores):
  qT/kT = W @ X.T computed directly in head-major layout (TF32 matmuls, PE at
  full rate), scoresT = k @ qT per head with 64x128 row-tiled matmul pairs (two
  heads run concurrently on the two halves of the PE array), one 2048-wide exp
  per 4-bank PSUM block on the scalar engine (the per-instruction overhead
  makes narrow activations 40% slower), AV as [v|1].T @ exps so the softmax
  denominators fall out of the matmul for free, normalization via vector-engine
  reciprocal + gpsimd partition-broadcast, then the output projection from the
  already-transposed attention output.
"""
import numpy as np

import concourse.bacc as bacc
import concourse.mybir as mybir
from concourse.tile import TileContext
from concourse.bass_utils import run_bass_kernel_spmd

P = 128
S = 2048          # sequence length
DM = 1024         # d_model
DG = 512          # dims per head-group (8 heads x 64)
NPAIR = 4         # head pairs per group
NQB = 4           # q blocks of 512
NKT = 16          # key tiles of 128
KT = DM // P      # contraction tiles for projections

F32 = mybir.dt.float32
F32R = mybir.dt.float32r
BF16 = mybir.dt.bfloat16
AF = mybir.ActivationFunctionType


def _build(has_bias):
    nc = bacc.Bacc(None, target_bir_lowering=False)
    xqT = nc.dram_tensor("xqT", [DM, S], F32R, kind="ExternalInput")
    xkT = nc.dram_tensor("xkT", [DM, S], F32R, kind="ExternalInput")
    xvT = nc.dram_tensor("xvT", [DM, S], F32R, kind="ExternalInput")
    wqT = nc.dram_tensor("wqT", [DM, DG], F32R, kind="ExternalInput")
    wkT = nc.dram_tensor("wkT", [DM, DG], F32R, kind="ExternalInput")
    wvT = nc.dram_tensor("wvT", [DM, DG], F32R, kind="ExternalInput")
    woT = nc.dram_tensor("woT", [DG, DM], F32R, kind="ExternalInput")
    if has_bias:
        bq = nc.dram_tensor("bq", [1, DG], F32R, kind="ExternalInput")
        bk = nc.dram_tensor("bk", [1, DG], F32R, kind="ExternalInput")
        bv = nc.dram_tensor("bv", [1, DG], F32R, kind="ExternalInput")
    out = nc.dram_tensor("out", [S, DM], F32, kind="ExternalOutput")

    xT = {"q": xqT, "k": xkT, "v": xvT}
    wT = {"q": wqT, "k": wkT, "v": wvT}

    with TileContext(nc) as tc:
        with tc.tile_pool(name="pres", bufs=1) as pres, \
             tc.tile_pool(name="px", bufs=6) as px, \
             tc.tile_pool(name="pex", bufs=2) as pex, \
             tc.tile_pool(name="psmall", bufs=2) as psmall, \
             tc.tile_pool(name="pout", bufs=2) as pout, \
             tc.tile_pool(name="ps_proj", bufs=2, space="PSUM") as ps_proj, \
             tc.tile_pool(name="ps_sc", bufs=2, space="PSUM") as ps_sc, \
             tc.tile_pool(name="ps_av", bufs=2, space="PSUM") as ps_av:

            # resident tensors
            wq_sb = pres.tile([P, KT, DG], F32R)
            wk_sb = pres.tile([P, KT, DG], F32R)
            wv_sb = pres.tile([P, KT, DG], F32R)
            wo_sb = pres.tile([P, NPAIR, DM], F32R)
            qT_sb = pres.tile([P, NPAIR, S], BF16)
            kT_sb = pres.tile([P, NPAIR, S], BF16)
            v_sb = pres.tile([P, NKT, 8, 65], BF16)
            attn_sb = pres.tile([P, NPAIR, S], F32R)
            w_sb = {"q": wq_sb, "k": wk_sb, "v": wv_sb}

            for t, d in ((wq_sb, wqT), (wk_sb, wkT), (wv_sb, wvT)):
                nc.sync.dma_start(t[:], d.rearrange("(kt p) n -> p kt n", p=P))
            nc.sync.dma_start(wo_sb[:], woT.rearrange("(kp p) o -> p kp o", p=P))
            nc.vector.memset(v_sb[:, :, :, 64:65], 1.0)

            if has_bias:
                x9 = pres.tile([P, DG], F32R)      # ones row, rest zero
                xv9 = pres.tile([P, P], F32R)
                w9 = {
                    "q": pres.tile([P, DG], F32R, name="w9q"),
                    "k": pres.tile([P, DG], F32R, name="w9k"),
                    "v": pres.tile([P, DG], F32R, name="w9v"),
                }
                for t in (x9, xv9, w9["q"], w9["k"], w9["v"]):
                    nc.vector.memset(t[:], 0.0)
                nc.vector.memset(x9[0:1, :], 1.0)
                nc.vector.memset(xv9[0:1, :], 1.0)
                for key, d in (("q", bq), ("k", bk), ("v", bv)):
                    nc.sync.dma_start(w9[key][0:1, :], d[:])

            emitted = set()

            def qk_block(proj, nb):
                """Project q or k for seq block nb, all 4 pairs: fills
                {q,k}T_sb[:, :, nb*512:(nb+1)*512]."""
                dst = qT_sb if proj == "q" else kT_sb
                xs = []
                for j in range(KT // 2):
                    xt = px.tile([P, 2, DG], F32R, tag="x",
                                 name=f"x_{proj}{nb}_{j}")
                    nc.sync.dma_start(
                        xt[:],
                        xT[proj].rearrange("(kt p) s -> p kt s", p=P)
                        [:, 2 * j:2 * j + 2, nb * DG:(nb + 1) * DG],
                    )
                    xs.append(xt)
                for p in range(NPAIR):
                    ps = ps_proj.tile([P, DG], F32, tag="pp", name=f"ps_{proj}{nb}_{p}")
                    for kt in range(KT):
                        nc.tensor.matmul(
                            ps[:], w_sb[proj][:, kt, p * P:(p + 1) * P],
                            xs[kt // 2][:, kt % 2, :],
                            start=(kt == 0), stop=(kt == KT - 1 and not has_bias),
                        )
                    if has_bias:
                        nc.tensor.matmul(
                            ps[:], w9[proj][:, p * P:(p + 1) * P], x9[:],
                            start=False, stop=True,
                        )
                    nc.vector.tensor_copy(dst[:, p, nb * DG:(nb + 1) * DG], ps[:])

            def v_block(m):
                """Project v for key tile m (128 positions, all 8 heads):
                fills v_sb[:, m, :, 0:64]."""
                xt = px.tile([P, KT, P], F32R, tag="x", name=f"xv{m}")
                nc.sync.dma_start(
                    xt[:],
                    xvT.rearrange("(kt p) s -> p kt s", p=P)
                    [:, :, m * P:(m + 1) * P],
                )
                ps = ps_proj.tile([P, DG], F32, tag="pp", name=f"ps_v{m}")
                for kt in range(KT):
                    nc.tensor.matmul(
                        ps[:], xt[:, kt, :], wv_sb[:, kt, :],
                        start=(kt == 0), stop=(kt == KT - 1 and not has_bias),
                    )
                if has_bias:
                    nc.tensor.matmul(ps[:], xv9[:], w9["v"][:],
                                     start=False, stop=True)
                nc.vector.tensor_copy(
                    v_sb[:, m, :, 0:64],
                    ps[:].rearrange("p (h d) -> p h d", d=64),
                )

            def ensure(key):
                if key in emitted:
                    return
                emitted.add(key)
                kind, idx = key
                if kind == "v":
                    v_block(idx)
                else:
                    qk_block(kind, idx)

            def oproj_chunk(qb):
                """Output projection for seq rows qb*512 .. qb*512+512."""
                for mi in range(4):
                    m = 4 * qb + mi
                    for n in range(2):
                        ps = ps_proj.tile([P, DG], F32, tag="pp", name=f"ps_o{m}_{n}")
                        for kp in range(NPAIR):
                            nc.tensor.matmul(
                                ps[:], attn_sb[:, kp, m * P:(m + 1) * P],
                                wo_sb[:, kp, n * DG:(n + 1) * DG],
                                start=(kp == 0), stop=(kp == NPAIR - 1),
                            )
                        ot = pout.tile([P, DG], F32, tag="ot",
                                       name=f"ot{m}_{n}")
                        nc.vector.tensor_copy(ot[:], ps[:])
                        nc.sync.dma_start(
                            out[m * P:(m + 1) * P, n * DG:(n + 1) * DG], ot[:])

            # attention: pair-outer, q-block, one key tile per group
            # (2-bank double-buffered scores tiles so exp(g) overlaps
            # scores(g+1) and the PE never sees a >1us gap)
            for p in range(NPAIR):
                for qb in range(NQB):
                    ensure(("q", qb))
                    if p == 0 and qb + 1 < NQB:
                        ensure(("q", qb + 1))
                    av = [
                        ps_av.tile([65, DG], F32, tag="av", name=f"av{p}_{qb}_{h}")
                        for h in range(2)
                    ]
                    for g in range(NKT):
                        ensure(("k", g // 4))
                        ensure(("v", g))
                        sc = ps_sc.tile([P, 2, DG], F32, tag="sc",
                                        name=f"sc{p}_{qb}_{g}")
                        for h in range(2):
                            nc.tensor.matmul(
                                sc[:, h, :],
                                kT_sb[64 * h:64 * h + 64, p,
                                      g * P:(g + 1) * P],
                                qT_sb[64 * h:64 * h + 64, p,
                                      qb * DG:(qb + 1) * DG],
                                start=True, stop=True,
                                tile_position=(64 * h, 0),
                            )
                        ex = pex.tile([P, 2, DG], BF16, tag="ex",
                                      name=f"ex{p}_{qb}_{g}")
                        nc.scalar.activation(ex[:], sc[:], AF.Exp, scale=0.125)
                        for h in range(2):
                            nc.tensor.matmul(
                                av[h][:],
                                v_sb[:, g, 2 * p + h, :],
                                ex[:, h, :],
                                start=(g == 0),
                                stop=(g == NKT - 1),
                            )
                    for h in range(2):
                        rr = psmall.tile([1, DG], F32, tag="rr",
                                         name=f"rr{p}_{qb}_{h}")
                        nc.vector.reciprocal(rr[0:1, :], av[h][64:65, :])
                        rbc = psmall.tile([P, DG], F32, tag="rbc",
                                          name=f"rbc{p}_{qb}_{h}")
                        nc.gpsimd.partition_broadcast(rbc[:], rr[0:1, :])
                        nc.vector.tensor_tensor(
                            attn_sb[64 * h:64 * h + 64, p,
                                    qb * DG:(qb + 1) * DG],
                            av[h][0:64, :],
                            rbc[0:64, :],
                            mybir.AluOpType.mult,
                        )
                    if p == NPAIR - 1:
                        oproj_chunk(qb)
    nc.compile()
    return nc


_CACHE = {}


def _get_nc(has_bias):
    if has_bias not in _CACHE:
        _CACHE[has_bias] = _build(has_bias)
    return _CACHE[has_bias]


def _tr(a):
    return np.ascontiguousarray(np.asarray(a, dtype=np.float32).T)


def _run(Q, K, V, Wq, bq, Wk, bk, Wv, bv, Wo, bo, trace=False):
    Q, K, V = (np.asarray(t, np.float32) for t in (Q, K, V))
    Wq, Wk, Wv, Wo = (np.asarray(t, np.float32) for t in (Wq, Wk, Wv, Wo))
    bq, bk, bv, bo = (np.asarray(t, np.float32) for t in (bq, bk, bv, bo))
    B = Q.shape[0]
    has_bias = bool(np.any(bq) or np.any(bk) or np.any(bv))
    nc = _get_nc(has_bias)

    xts = [(_tr(Q[b]), _tr(K[b]), _tr(V[b])) for b in range(B)]
    wts = []
    for g in range(2):
        sl = slice(DG * g, DG * (g + 1))
        wts.append({
            "wqT": _tr(Wq[sl]), "wkT": _tr(Wk[sl]), "wvT": _tr(Wv[sl]),
            "woT": _tr(Wo[:, sl]),
            "bq": np.ascontiguousarray(bq[None, sl]),
            "bk": np.ascontiguousarray(bk[None, sl]),
            "bv": np.ascontiguousarray(bv[None, sl]),
        })
    in_maps = []
    for c in range(8):
        b, g = c // 2, c % 2
        m = {
            "xqT": xts[b][0], "xkT": xts[b][1], "xvT": xts[b][2],
            "wqT": wts[g]["wqT"], "wkT": wts[g]["wkT"],
            "wvT": wts[g]["wvT"], "woT": wts[g]["woT"],
        }
        if has_bias:
            m["bq"] = wts[g]["bq"]
            m["bk"] = wts[g]["bk"]
            m["bv"] = wts[g]["bv"]
        in_maps.append(m)

    res = run_bass_kernel_spmd(nc, in_maps, core_ids=list(range(8)),
                               trace=trace)
    outp = np.empty((B, S, DM), np.float32)
    for b in range(B):
        outp[b] = res.results[2 * b]["out"] + res.results[2 * b + 1]["out"]
    outp += bo[None, None, :]
    return outp, res


def kernel(Q, K, V, Wq, bq, Wk, bk, Wv, bv, Wo, bo):
    outp, _ = _run(Q, K, V, Wq, bq, Wk, bk, Wv, bv, Wo, bo, trace=False)
    return outp
